# revision 1
# baseline (speedup 1.0000x reference)
"""Trainium2 Bass kernel for nn_MechanicsFunctionsMultiBlock.

Computes per-element hessians of a Neo-Hookean energy (linear triangles,
one quadrature point) for 800k elements split into two material blocks.

Sharding (hardcoded per spec): elements are sharded across the 8
NeuronCores by material block — cores 0-3 take quarters of blocks0
(lam=1.0, mu=0.5), cores 4-7 quarters of blocks1 (lam=2.0, mu=1.0).
Per-element rows (shapeGrads / vols / state / conns-gathered U rows) are
gathered on the host while sharding; the output element-hessian array
stays sharded along the element axis so the final scatter is a plain
per-core block write.

Closed form used on device (validated to ~6e-3 L2 in bf16):
  G = shapeGrads[e,0]  (3x2),  u = U[conns[e]]  (3x2)
  gradU = u^T G,  F = I + gradU,  J = det F,  lnJ = ln J
  ghat = G adj(F)          (= J * G F^-1, no division)
  c1 = mu (1 + 0.01 q),  c2 = c1 - lam lnJ
  x = (vol / J^2) ghat,  S[n,m] = vol c1 (G G^T)[n,m]
  H[n,a,m,b] = S[n,m] d_ab + c2 x[n,b] ghat[m,a] + lam x[n,a] ghat[m,b]

Device schedule (one 128x782 SoA chunk per core, all bf16):
  bf16 end-to-end: DVE tensor_tensor runs in 2x_1P mode (half the fp32
  stream time) and all HBM traffic is halved. Only the 21 unique
  hessian planes are written (the 15 symmetric duplicates are
  reconstructed on the host), so output DMA is 21x128x782x2B = 4.2 MB
  per core vs 14.4 MB for the fp32/36-plane variant. All dram tensors
  are partition-major ([128, planes, 782]) so every DMA is a clean
  per-partition contiguous run with no rearrange APs; inputs ride in
  one merged 14-plane tensor (fewer per-launch buffers).
  J is computed cancellation-free for bf16: j1 = J - 1 =
  (gu00+gu11) + (gu00*gu11 - gu01*gu10) stays O(gradU) so bf16 keeps
  ~8 significant bits on it; lnJ = Ln(j1 + 1) rides the ACT engine's
  fp32-internal affine. Elementwise planes live in one 55-plane SBUF
  arena; the 21 output planes are overlaid on the input planes.
  The DVE is the bottleneck (~121 TT plane traversals; CoreSim span
  ~62.6 us/core with the DVE gap-free through its whole window), so
  plane groups are fused into wide multi-plane ops via strided /
  broadcast / alternating / negative-stride APs (gradU products and
  ghat each collapse to ~3 six-plane ops; the lam-swap uses a 4x-mode
  tensor_scalar over swapped pairs; scalar_tensor_tensor was tried and
  reverted - TSP has no 2x uop). The diagonal GG^T squares ride the
  idle ACT engine (Square LUT); G planes are i-major so the i=0 trio
  loads first and stage-1 starts after 0.6 MB of input. The +S entries
  (po 6..20) are computed before the off-diagonal planes so their DMA
  waves overlap compute, leaving a single 6-plane wave in the tail.
  GPSIMD offload and SWDGE/CCE accumulate-DMA were rejected: GpSimd
  shares an exclusive-lock SBUF port pair with the DVE's second read
  port, so both starve while the TT stream runs.
"""
import numpy as np
import ml_dtypes

import concourse.bass as bass
import concourse.tile as tile
from concourse import mybir
from concourse.bass_utils import run_bass_kernel_spmd
from concourse.vector_clock import ScopedClock, VectorClock

# ---------------------------------------------------------------- constants
E = 800_000
N = 400_000
MATS = ((1.0, 0.5), (2.0, 1.0))  # (lam, mu) for block0 / block1
NCORES = 8
K = E // 2 // 4            # 100_000 elements per core
PART = 128
FREE = 782                 # 128*782 = 100_096 padded elements per core (min legal)
ELP = PART * FREE

F32 = mybir.dt.float32
BF16 = mybir.dt.bfloat16
BF16NP = ml_dtypes.bfloat16
ALU = mybir.AluOpType
ACTF = mybir.ActivationFunctionType

# ---- output plane order (also the arena slot order, po = slots 0..20) ----
# 6x6 hessian entry (r, c): r = 2n + a, c = 2m + b. The 21 planes cover
# the upper triangle (15 strict + 6 diagonal); the host writes each
# strict-upper plane to both (r,c) and (c,r).
_NM = [(0, 1), (0, 2), (1, 2)]
PO_ORDER = (
    [rc for (n, m) in _NM for rc in ((2 * n, 2 * m + 1), (2 * n + 1, 2 * m))]
    + [(2 * n, 2 * n + 1) for n in range(3)]
    + [(2 * n + a, 2 * m + a) for (n, m) in _NM for a in range(2)]
    + [(r, r) for r in range(6)]
)
_PLANE = {rc: i for i, rc in enumerate(PO_ORDER)}

# ---- arena slot map (55 planes of [128, 782] bf16) -----------------------
# 0..20  : po (21 output planes); 0..7 double as fin (g00..g21, vol, q),
#          8..13 double as u6 — inputs are fully consumed before the
#          first H plane is written.
# 21..24 : gu (gradU)   25..31 : misc   32..37 : ghat
# 38..43 : S            44..49 : x (→ x2 in place)   50..54 : temps
NSLOT = 55
_GU, _MISC, _GH, _SS, _X, _TMP = 21, 25, 32, 38, 44, 50


def _split_drain(tc_cls):
    """TileContext whose tail drain emits one sem wait per no-op.

    The walrus build here rejects instructions carrying more than one
    sync wait; TileContext's stock exit puts every live processor's
    final tick on a single Drain.
    """

    class SplitDrainTileContext(tc_cls):
        def _drain_and_barrier(self, tick_clock, wait_clock):
            ticks = list(tick_clock.global_clock)
            for i, t in enumerate(ticks):
                if t <= 0:
                    continue
                sub = [t if j == i else 0 for j in range(len(ticks))]
                nop = self.nc.sync.nop()
                wait_clock.add_sem_waits(nop.ins, ScopedClock({None: VectorClock(sub)}))
            self.nc.sync.drain()
            self.nc.all_engine_barrier()
            assert self.sems is not None
            popped = self.nc._tile_sem_poison_stack.pop()
            assert popped is self._sem_poison
            self.nc.clear_and_free_semaphores(list(self.sems.allocated().values()))
            self.nc.all_engine_barrier()

    return SplitDrainTileContext


def _legalize_single_wait(nc):
    """Split multi-wait instructions: this walrus build encodes at most one
    sync wait per instruction (two for EventSemaphore). Hoist extra waits
    onto same-engine no-ops inserted immediately before."""
    import bass_rust

    n = 0
    for fn in nc.m.functions:
        for blk in fn.blocks:
            out = []
            for ins in blk.instructions:
                si = ins.sync_info
                cap = 2 if isinstance(ins, mybir.InstEventSemaphore) else 1
                if si is not None and len(si.on_wait) > cap:
                    waits = list(si.on_wait)
                    for w in waits[:-cap]:
                        nop = mybir.InstNoOp(name=f"I-wsplit-{n}", ins=[], outs=[])
                        n += 1
                        nop.engine = ins.engine
                        nop.sync_info = bass_rust.SyncInfo(on_wait=[w], on_update=[])
                        out.append(nop)
                    ins.sync_info = bass_rust.SyncInfo(
                        on_wait=waits[-cap:], on_update=list(si.on_update)
                    )
                out.append(ins)
            blk.instructions = out


# ------------------------------------------------------------- bass program
def build_nc(legalize=True, variant="full", repeat=1):
    lvl = {"dma": 0, "full": 3}[variant]
    nc = bass.Bass()
    # single merged input tensor: planes 0..5 G, 6 vol, 7 q, 8..13 u6
    fin = nc.declare_dram_parameter("fin", [PART, 14, FREE], BF16, isOutput=False)
    mats = nc.declare_dram_parameter("mats", [PART, 4], F32, isOutput=False)
    hout = nc.declare_dram_parameter("hout", [PART, 21, FREE], BF16, isOutput=True)

    TC = _split_drain(tile.TileContext)
    with TC(nc) as tc:
        with tc.tile_pool(name="arena_pool", bufs=1) as pool:
            mt = pool.tile([PART, 4], F32, name="mats_t", tag="mats_t")
            ar = pool.tile([PART, NSLOT, FREE], BF16, name="arena", tag="arena")

            def sl(i):
                return ar[:, i, :]

            # -------- input DMAs: fin planes -> slots 0..13 in four waves.
            # G planes are i-major (slot 3i+n); the i=0 trio loads first so
            # stage-1 work starts after only 0.6 MB.
            nc.sync.dma_start(out=ar[:, 0:3, :], in_=fin[:, 0:3, :])
            nc.sync.dma_start(out=ar[:, 3:6, :], in_=fin[:, 3:6, :])
            nc.sync.dma_start(out=ar[:, 8:14, :], in_=fin[:, 8:14, :])
            nc.sync.dma_start(out=ar[:, 6:8, :], in_=fin[:, 6:8, :])
            nc.sync.dma_start(out=mt[:], in_=mats[:])
            ap_lam, ap_mu = mt[:, 0:1], mt[:, 1:2]
            ap_mu001, ap_nlam = mt[:, 2:3], mt[:, 3:4]

            g = lambda n, i: sl(3 * i + n)
            vol, q = sl(6), sl(7)
            u = lambda n, cc: sl(8 + 2 * n + cc)
            h = lambda n, a, m, b: sl(_PLANE[(2 * n + a, 2 * m + b)])
            gu = lambda i: sl(_GU + i)
            ghat = lambda n, a: sl(_GH + 2 * n + a)
            # off-diagonal S entries first, diagonal last: lets the stage-6/7
            # "+S" adds run as ONE 12-plane op (in1 = [[FREE,6],[0,2]])
            PAIRS = [(0, 1), (0, 2), (1, 2), (0, 0), (1, 1), (2, 2)]
            S = lambda n, m: sl(_SS + PAIRS.index((min(n, m), max(n, m))))
            x = lambda n, a: sl(_X + 2 * n + a)
            F00, F11, J, lnJ, iJ2, c1, c2 = (sl(_MISC + i) for i in range(7))
            c2l, w2, vc1 = J, iJ2, c1  # slot reuse after J/iJ2/c1 die
            t0, P1, P2, q12a, q12b = (sl(_TMP + i) for i in range(5))

            TT = nc.vector.tensor_tensor
            STT = nc.vector.scalar_tensor_tensor
            ACT = nc.scalar.activation

            import dataclasses as _dc

            def pl(base, count, step=1):
                a = sl(base)
                return _dc.replace(a, ap=[a.ap[0], [step * FREE, count], [1, FREE]])

            def bc(plane, count):
                return _dc.replace(plane, ap=[plane.ap[0], [0, count], plane.ap[1]])

            def rpl(base, count):
                # like pl() but walking slots downward (negative mid stride)
                a = sl(base)
                return _dc.replace(a, ap=[a.ap[0], [-FREE, count], [1, FREE]])

            def _full_body():
                # ---- 1. S'[n,m] = (G G^T)[n,m] (needs G planes only;
                #         overlaps the vol/q/u6 input DMAs). The diagonal
                #         squares ride the idle ACT engine (Square LUT);
                #         off-diagonal pairs fuse over consecutive i-major
                #         G slots.
                # sq1 lands in the ghat region (free until stage 4) so the
                # gradU temps in _X don't serialize behind the ACT squares.
                # Diagonal S' -> slots _SS+3..5, off-diagonal -> _SS..+2.
                ACT(out=pl(_SS + 3, 3), in_=pl(0, 3), func=ACTF.Square)
                ACT(out=pl(_GH, 3), in_=pl(3, 3), func=ACTF.Square)
                TT(out=pl(_SS + 3, 3), in0=pl(_SS + 3, 3), in1=pl(_GH, 3), op=ALU.add)
                # off-diag (0,1),(0,2): g(m,i) now consecutive slots
                TT(out=pl(_SS, 2), in0=bc(g(0, 0), 2), in1=pl(1, 2),
                   op=ALU.mult)
                TT(out=pl(_TMP, 2), in0=bc(g(0, 1), 2), in1=pl(4, 2),
                   op=ALU.mult)
                TT(out=pl(_SS, 2), in0=pl(_SS, 2), in1=pl(_TMP, 2), op=ALU.add)
                # off-diag (1,2)
                d = sl(_SS + 2)
                TT(out=d, in0=g(1, 0), in1=g(2, 0), op=ALU.mult)
                TT(out=t0, in0=g(1, 1), in1=g(2, 1), op=ALU.mult)
                TT(out=d, in0=d, in1=t0, op=ALU.add)

                # ---- 2. gradU: per column one 6-plane product (u broadcast
                #         over i), then 2-plane tree adds; temps in x region
                for cc in range(2):
                    ucc = sl(8 + cc)
                    g0 = sl(0)
                    xb = sl(_X)
                    TT(out=_dc.replace(xb, ap=[xb.ap[0], [3 * FREE, 2], [FREE, 3], [1, FREE]]),
                       in0=_dc.replace(ucc, ap=[ucc.ap[0], [0, 2], [2 * FREE, 3], [1, FREE]]),
                       in1=_dc.replace(g0, ap=[g0.ap[0], [3 * FREE, 2], [FREE, 3], [1, FREE]]),
                       op=ALU.mult)
                    TT(out=pl(_GU + 2 * cc, 2), in0=pl(_X, 2, 3),
                       in1=pl(_X + 1, 2, 3), op=ALU.add)
                    TT(out=pl(_GU + 2 * cc, 2), in0=pl(_GU + 2 * cc, 2),
                       in1=pl(_X + 2, 2, 3), op=ALU.add)
                gu00, gu01, gu10, gu11 = gu(0), gu(1), gu(2), gu(3)

                # ---- 3. F planes for ghat; cancellation-free j1 = J - 1;
                #         transcendentals on ACT (fp32 internal affine keeps
                #         the +1 exact). t0,P1 = (gu00*gu11, gu01*gu10) in
                #         one paired op (in0 slots 21,22; in1 slots 24,23).
                ACT(out=F00, in_=gu00, func=ACTF.Identity, bias=1.0)
                ACT(out=F11, in_=gu11, func=ACTF.Identity, bias=1.0)
                TT(out=pl(_TMP, 2), in0=pl(_GU, 2), in1=rpl(_GU + 3, 2), op=ALU.mult)
                TT(out=P2, in0=gu00, in1=gu11, op=ALU.add)
                TT(out=t0, in0=t0, in1=P1, op=ALU.subtract)
                TT(out=J, in0=P2, in1=t0, op=ALU.add)      # J slot holds j1
                ACT(out=lnJ, in_=J, func=ACTF.Ln, bias=1.0)
                ACT(out=iJ2, in_=lnJ, func=ACTF.Exp, scale=-2.0)
                ACT(out=c1, in_=q, func=ACTF.Identity, scale=ap_mu001, bias=ap_mu)
                ACT(out=c2, in_=lnJ, func=ACTF.Copy, scale=ap_nlam)
                TT(out=c2, in0=c2, in1=c1, op=ALU.add)
                # j1 dead (lnJ taken); c2l shares its slot
                ACT(out=c2l, in_=c2, func=ACTF.Identity, bias=ap_lam)

                # ---- 4. ghat = G adj(F) in three 6-plane ops: ghat(n,a) =
                #         g(n,a)*Falt - g(n,1-a)*gualt with Falt/gualt
                #         alternating-broadcast APs ((F11,F00), (gu10,gu01));
                #         G is i-major so the a-dim walks in 3-slot strides.
                g0, g3 = sl(0), sl(3)
                gho = sl(_GH)
                xb = sl(_X)
                TT(out=_dc.replace(gho, ap=[gho.ap[0], [2 * FREE, 3], [FREE, 2], [1, FREE]]),
                   in0=_dc.replace(g0, ap=[g0.ap[0], [FREE, 3], [3 * FREE, 2], [1, FREE]]),
                   in1=_dc.replace(F11, ap=[F11.ap[0], [0, 3], [-FREE, 2], [1, FREE]]),
                   op=ALU.mult)
                TT(out=_dc.replace(xb, ap=[xb.ap[0], [2 * FREE, 3], [FREE, 2], [1, FREE]]),
                   in0=_dc.replace(g3, ap=[g3.ap[0], [FREE, 3], [-3 * FREE, 2], [1, FREE]]),
                   in1=_dc.replace(gu10, ap=[gu10.ap[0], [0, 3], [-FREE, 2], [1, FREE]]),
                   op=ALU.mult)
                TT(out=pl(_GH, 6), in0=pl(_GH, 6), in1=pl(_X, 6), op=ALU.subtract)

                # ---- 5. (w2, vc1) = vol * (iJ2, c1) paired in place, then
                #         ONE 12-plane op: group0 x = w2*ghat (ghat 32..37 ->
                #         x 44..49), group1 S = vc1*S' (38..43 in place);
                #         in0 spans ghat|S' consecutively, in1 broadcasts
                #         (w2, vc1) from consecutive slots 29..30.
                TT(out=pl(_MISC + 4, 2), in0=bc(vol, 2), in1=pl(_MISC + 4, 2),
                   op=ALU.mult)
                gh0 = sl(_GH)
                xo = sl(_X)
                w2s = sl(_MISC + 4)
                TT(out=_dc.replace(xo, ap=[xo.ap[0], [-6 * FREE, 2], [FREE, 6], [1, FREE]]),
                   in0=_dc.replace(gh0, ap=[gh0.ap[0], [6 * FREE, 2], [FREE, 6], [1, FREE]]),
                   in1=_dc.replace(w2s, ap=[w2s.ap[0], [FREE, 2], [0, 6], [1, FREE]]),
                   op=ALU.mult)

                # ---- 6. x2 = c2l x -> slots 21..26 (gu/F dead); the +S
                #         entries (po 6..20) are computed FIRST so their
                #         waves overlap the off-diagonal stage below, leaving
                #         only one 6-plane wave in the tail.
                X2 = 21
                TT(out=pl(X2, 6), in0=bc(c2l, 6), in1=pl(_X, 6), op=ALU.mult)

                # H[n,0,n,1] = x2[n,0] ghat[n,1]   (po slots 6..8)
                TT(out=pl(6, 3), in0=pl(X2, 3, 2), in1=pl(_GH + 1, 3, 2), op=ALU.mult)
                # wave 1: slots 6..8 ship immediately
                nc.sync.dma_start(out=hout[:, 6:9, :], in_=ar[:, 6:9, :])
                # a == b, n < m: H = x2[n,a] ghat[m,a] + S[n,m]  (po 9..14)
                # products: (n,m)=(0,1),(0,2) share x2[0,a] -> one 4-plane op
                x20 = sl(X2)
                p9o = sl(9)
                TT(out=_dc.replace(p9o, ap=[p9o.ap[0], [2 * FREE, 2], [FREE, 2], [1, FREE]]),
                   in0=_dc.replace(x20, ap=[x20.ap[0], [0, 2], [FREE, 2], [1, FREE]]),
                   in1=_dc.replace(sl(_GH + 2), ap=[sl(_GH + 2).ap[0], [2 * FREE, 2], [FREE, 2], [1, FREE]]),
                   op=ALU.mult)
                TT(out=pl(13, 2), in0=pl(X2 + 2, 2), in1=pl(_GH + 4, 2), op=ALU.mult)

                # ---- 7. diagonal products: H[n,a,n,a] = x2[n,a] ghat[n,a]
                TT(out=pl(15, 6), in0=pl(X2, 6), in1=pl(_GH, 6), op=ALU.mult)
                # merged adds: slots 9..20 += S, S enumerating
                # (S01,S01,S02,S02,S12,S12,S00,S00,S11,S11,S22,S22)
                p9 = sl(9)
                s0 = sl(_SS)
                TT(out=_dc.replace(p9, ap=[p9.ap[0], [2 * FREE, 6], [FREE, 2], [1, FREE]]),
                   in0=_dc.replace(p9, ap=[p9.ap[0], [2 * FREE, 6], [FREE, 2], [1, FREE]]),
                   in1=_dc.replace(s0, ap=[s0.ap[0], [FREE, 6], [0, 2], [1, FREE]]),
                   op=ALU.add)
                # wave 2: slots 9..20 in one transfer (overlaps the stage below)
                nc.sync.dma_start(out=hout[:, 9:21, :], in_=ar[:, 9:21, :])

                # ---- 8. off-diagonal pairs (po slots 0..5): three raw
                #         product pairs (P1,P2) land directly in po 2k,2k+1;
                #         one 6-plane swapped-pair tensor_scalar makes
                #         lam*(P2,P1) in slots 38..43 (S dead); then one
                #         6-plane *c2 in place and one 6-plane add.
                for k, (n, m) in enumerate(_NM):
                    TT(out=pl(2 * k, 2), in0=rpl(_X + 2 * n + 1, 2),
                       in1=pl(_GH + 2 * m, 2), op=ALU.mult)
                p1b = sl(1)
                nc.vector.tensor_scalar_mul(
                    out=pl(_SS, 6),
                    in0=_dc.replace(p1b, ap=[p1b.ap[0], [2 * FREE, 3], [-FREE, 2], [1, FREE]]),
                    scalar1=ap_lam)
                TT(out=pl(0, 6), in0=pl(0, 6), in1=bc(c2, 6), op=ALU.mult)
                TT(out=pl(0, 6), in0=pl(0, 6), in1=pl(_SS, 6), op=ALU.add)

                # wave 4 (tail): off-diagonal planes
                nc.sync.dma_start(out=hout[:, 0:6, :], in_=ar[:, 0:6, :])

            def _dma_body():
                nc.vector.memset(ar[:, 0:21, :], 0.0)
                nc.sync.dma_start(out=hout[:, 0:21, :], in_=ar[:, 0:21, :])

            for _rep in range(repeat):
                if lvl >= 3:
                    _full_body()
                else:
                    _dma_body()
    if legalize:
        _legalize_single_wait(nc)
    return nc


_NC_CACHE = None


def _get_nc():
    global _NC_CACHE
    if _NC_CACHE is None:
        _NC_CACHE = build_nc()
    return _NC_CACHE


# ------------------------------------------------------------------- host
def _shard_core(U, state, conns, shapeGrads, vols, ids, lam, mu):
    KX = len(ids)
    g6 = shapeGrads[ids, 0]                          # [KX, 3, 2] (n, i)
    fin = np.zeros((14, ELP), np.float32)
    fin[:6, :KX] = g6.transpose(2, 1, 0).reshape(6, KX)   # plane 3i+n
    fin[6, :KX] = vols[ids, 0]
    fin[7, :KX] = state[ids, 0, 0]
    uu = U[conns[ids]].reshape(KX, 6)               # (n,c) C-order
    fin[8:14, :KX] = uu.T

    mats = np.empty((PART, 4), np.float32)
    mats[:, 0] = lam
    mats[:, 1] = mu
    mats[:, 2] = 0.01 * mu
    mats[:, 3] = -lam
    return {
        "fin": np.ascontiguousarray(
            fin.reshape(14, PART, FREE).transpose(1, 0, 2)).astype(BF16NP),
        "mats": mats,
    }


_R21 = np.array([rc[0] for rc in PO_ORDER])
_C21 = np.array([rc[1] for rc in PO_ORDER])


def _decode_core(hout):
    # hout [128, 21, 784] bf16, partition-major
    planes = np.asarray(hout).astype(np.float32).transpose(1, 0, 2).reshape(21, ELP)[:, :K]
    Hm = np.empty((K, 6, 6), np.float32)
    Hm[:, _R21, _C21] = planes.T
    Hm[:, _C21[:15], _R21[:15]] = planes[:15].T     # symmetric duplicates
    return Hm.reshape(K, 3, 2, 3, 2)


def kernel(**inputs):
    U = np.asarray(inputs["U"], np.float32)
    state = np.asarray(inputs["state"], np.float32)
    conns = np.asarray(inputs["conns"])
    shapeGrads = np.asarray(inputs["shapeGrads"], np.float32)
    vols = np.asarray(inputs["vols"], np.float32)
    blocks = (np.asarray(inputs["blocks0"]), np.asarray(inputs["blocks1"]))

    core_ids = list(range(NCORES))
    in_maps = []
    id_lists = []
    for d in core_ids:
        blk, (lam, mu) = blocks[d // 4], MATS[d // 4]
        ids = blk[(d % 4) * K : (d % 4 + 1) * K]
        id_lists.append(ids)
        in_maps.append(_shard_core(U, state, conns, shapeGrads, vols, ids, lam, mu))

    res = run_bass_kernel_spmd(_get_nc(), in_maps, core_ids=core_ids)

    hess = np.empty((E, 3, 2, 3, 2), np.float32)
    for d in core_ids:
        hess[id_lists[d]] = _decode_core(res.results[d]["hout"])
    return hess



# revision 11
# speedup vs baseline: 7.1265x; 7.1265x over previous
"""Trainium2 Bass kernel for nn_MechanicsFunctionsMultiBlock.

Computes per-element hessians of a Neo-Hookean energy (linear triangles,
one quadrature point) for 800k elements split into two material blocks.

Sharding (hardcoded per spec): elements are sharded across the 8
NeuronCores by material block — cores 0-3 take quarters of blocks0
(lam=1.0, mu=0.5), cores 4-7 quarters of blocks1 (lam=2.0, mu=1.0).
Per-element rows (shapeGrads / vols / state / conns-gathered U rows) are
gathered on the host while sharding; the output element-hessian array
stays sharded along the element axis so the final scatter is a plain
per-core block write.

Closed form used on device (validated to ~6e-3 L2 in bf16):
  G = shapeGrads[e,0]  (3x2),  u = U[conns[e]]  (3x2)
  gradU = u^T G,  F = I + gradU,  J = det F,  lnJ = ln J
  ghat = G adj(F)          (= J * G F^-1, no division)
  c1 = mu (1 + 0.01 q),  c2 = c1 - lam lnJ
  x = (vol / J^2) ghat,  S[n,m] = vol c1 (G G^T)[n,m]
  H[n,a,m,b] = S[n,m] d_ab + c2 x[n,b] ghat[m,a] + lam x[n,a] ghat[m,b]

Device schedule (one 128x782 SoA chunk per core, all bf16):
  bf16 end-to-end: DVE tensor_tensor runs in 2x_1P mode (half the fp32
  stream time) and all HBM traffic is halved. Only the 21 unique
  hessian planes are written (the 15 symmetric duplicates are
  reconstructed on the host), so output DMA is 21x128x782x2B = 4.2 MB
  per core vs 14.4 MB for the fp32/36-plane variant. All dram tensors
  are partition-major ([128, planes, 782]) so every DMA is a clean
  per-partition contiguous run with no rearrange APs; inputs ride in
  one merged 14-plane tensor (fewer per-launch buffers).
  J is computed cancellation-free for bf16: j1 = J - 1 =
  (gu00+gu11) + (gu00*gu11 - gu01*gu10) stays O(gradU) so bf16 keeps
  ~8 significant bits on it; lnJ = Ln(j1 + 1) rides the ACT engine's
  fp32-internal affine. Elementwise planes live in one 55-plane SBUF
  arena; the 21 output planes are overlaid on the input planes.
  The DVE is the bottleneck (~121 TT plane traversals; CoreSim span
  ~62.6 us/core with the DVE gap-free through its whole window), so
  plane groups are fused into wide multi-plane ops via strided /
  broadcast / alternating / negative-stride APs (gradU products and
  ghat each collapse to ~3 six-plane ops; the lam-swap uses a 4x-mode
  tensor_scalar over swapped pairs; scalar_tensor_tensor was tried and
  reverted - TSP has no 2x uop). The diagonal GG^T squares ride the
  idle ACT engine (Square LUT); G planes are i-major so the i=0 trio
  loads first and stage-1 starts after 0.6 MB of input. The +S entries
  (po 6..20) are computed before the off-diagonal planes so their DMA
  waves overlap compute, leaving a single 6-plane wave in the tail.
  GPSIMD offload and SWDGE/CCE accumulate-DMA were rejected: GpSimd
  shares an exclusive-lock SBUF port pair with the DVE's second read
  port, so both starve while the TT stream runs.
"""
import numpy as np
import ml_dtypes

import concourse.bass as bass
import concourse.tile as tile
from concourse import mybir
from concourse.bass_utils import run_bass_kernel_spmd
from concourse.vector_clock import ScopedClock, VectorClock

# ---------------------------------------------------------------- constants
E = 800_000
N = 400_000
MATS = ((1.0, 0.5), (2.0, 1.0))  # (lam, mu) for block0 / block1
NCORES = 8
K = E // 2 // 4            # 100_000 elements per core
PART = 128
FREE = 782                 # 128*782 = 100_096 padded elements per core (min legal)
ELP = PART * FREE

F32 = mybir.dt.float32
BF16 = mybir.dt.bfloat16
BF16NP = ml_dtypes.bfloat16
ALU = mybir.AluOpType
ACTF = mybir.ActivationFunctionType

# ---- output plane order (also the arena slot order, po = slots 0..20) ----
# 6x6 hessian entry (r, c): r = 2n + a, c = 2m + b. The 21 planes cover
# the upper triangle (15 strict + 6 diagonal); the host writes each
# strict-upper plane to both (r,c) and (c,r).
_NM = [(0, 1), (0, 2), (1, 2)]
PO_ORDER = (
    [rc for (n, m) in _NM for rc in ((2 * n, 2 * m + 1), (2 * n + 1, 2 * m))]
    + [(2 * n, 2 * n + 1) for n in range(3)]
    + [(2 * n + a, 2 * m + a) for (n, m) in _NM for a in range(2)]
    + [(r, r) for r in range(6)]
)
_PLANE = {rc: i for i, rc in enumerate(PO_ORDER)}

# ---- arena slot map (61 planes of [128, 782] bf16) -----------------------
# 0..20  : po (21 output planes); 0..7 double as fin (g00..g21, vol, q),
#          8..13 double as u6 — inputs are fully consumed before the
#          first H plane is written.
# 21..24 : gu (gradU)   25..31 : misc   32..37 : ghat
# 38..43 : S            44..49 : x (→ x2 in place)   50..54 : temps
# 55..57 : Pool scratch (i=1 off-diag S partial products)
# 58..60 : ACT scratch (i=1 diagonal squares)
NSLOT = 61
_GU, _MISC, _GH, _SS, _X, _TMP = 21, 25, 32, 38, 44, 50
_PP, _PQ = 55, 58


def _split_drain(tc_cls):
    """TileContext whose tail drain emits one sem wait per no-op.

    The walrus build here rejects instructions carrying more than one
    sync wait; TileContext's stock exit puts every live processor's
    final tick on a single Drain.
    """

    class SplitDrainTileContext(tc_cls):
        def _drain_and_barrier(self, tick_clock, wait_clock):
            ticks = list(tick_clock.global_clock)
            for i, t in enumerate(ticks):
                if t <= 0:
                    continue
                sub = [t if j == i else 0 for j in range(len(ticks))]
                nop = self.nc.sync.nop()
                wait_clock.add_sem_waits(nop.ins, ScopedClock({None: VectorClock(sub)}))
            self.nc.sync.drain()
            self.nc.all_engine_barrier()
            assert self.sems is not None
            popped = self.nc._tile_sem_poison_stack.pop()
            assert popped is self._sem_poison
            self.nc.clear_and_free_semaphores(list(self.sems.allocated().values()))
            self.nc.all_engine_barrier()

    return SplitDrainTileContext


def _legalize_single_wait(nc):
    """Split multi-wait instructions: this walrus build encodes at most one
    sync wait per instruction (two for EventSemaphore). Hoist extra waits
    onto same-engine no-ops inserted immediately before."""
    import bass_rust

    n = 0
    for fn in nc.m.functions:
        for blk in fn.blocks:
            out = []
            for ins in blk.instructions:
                si = ins.sync_info
                cap = 2 if isinstance(ins, mybir.InstEventSemaphore) else 1
                if si is not None and len(si.on_wait) > cap:
                    waits = list(si.on_wait)
                    for w in waits[:-cap]:
                        nop = mybir.InstNoOp(name=f"I-wsplit-{n}", ins=[], outs=[])
                        n += 1
                        nop.engine = ins.engine
                        nop.sync_info = bass_rust.SyncInfo(on_wait=[w], on_update=[])
                        out.append(nop)
                    ins.sync_info = bass_rust.SyncInfo(
                        on_wait=waits[-cap:], on_update=list(si.on_update)
                    )
                out.append(ins)
            blk.instructions = out


# ------------------------------------------------------------- bass program
def build_nc(legalize=True, variant="full", repeat=1):
    lvl = {"dma": 0, "full": 3}[variant]
    nc = bass.Bass()
    # single merged input tensor: planes 0..5 G, 6 vol, 7 q, 8..13 u6
    fin = nc.declare_dram_parameter("fin", [PART, 14, FREE], BF16, isOutput=False)
    mats = nc.declare_dram_parameter("mats", [PART, 4], F32, isOutput=False)
    hout = nc.declare_dram_parameter("hout", [PART, 21, FREE], BF16, isOutput=True)

    TC = _split_drain(tile.TileContext)
    with TC(nc) as tc:
        # bufs=2: consecutive repeats alternate between two SBUF arenas so
        # iteration k+1's input DMAs overlap iteration k's compute (the
        # steady-state per-exec time is then max per-engine busy, not the
        # serial head+compute+tail span). With repeat=1 this changes nothing.
        with tc.tile_pool(name="arena_pool", bufs=min(2, repeat)) as pool:

          def _emit_rep():
            mt = pool.tile([PART, 4], F32, name="mats_t", tag="mats_t")
            ar = pool.tile([PART, NSLOT, FREE], BF16, name="arena", tag="arena")

            def sl(i):
                return ar[:, i, :]

            # -------- input DMAs: fin planes -> slots 0..13 in four waves.
            # G planes are i-major (slot 3i+n); the i=0 trio loads first so
            # stage-1 work starts after only 0.6 MB.
            nc.sync.dma_start(out=ar[:, 0:3, :], in_=fin[:, 0:3, :])
            nc.sync.dma_start(out=ar[:, 3:6, :], in_=fin[:, 3:6, :])
            nc.sync.dma_start(out=ar[:, 8:14, :], in_=fin[:, 8:14, :])
            nc.sync.dma_start(out=ar[:, 6:8, :], in_=fin[:, 6:8, :])
            nc.sync.dma_start(out=mt[:], in_=mats[:])
            ap_lam, ap_mu = mt[:, 0:1], mt[:, 1:2]
            ap_mu001, ap_nlam = mt[:, 2:3], mt[:, 3:4]

            g = lambda n, i: sl(3 * i + n)
            vol, q = sl(6), sl(7)
            u = lambda n, cc: sl(8 + 2 * n + cc)
            h = lambda n, a, m, b: sl(_PLANE[(2 * n + a, 2 * m + b)])
            gu = lambda i: sl(_GU + i)
            ghat = lambda n, a: sl(_GH + 2 * n + a)
            # off-diagonal S entries first, diagonal last: lets the stage-6/7
            # "+S" adds run as ONE 12-plane op (in1 = [[FREE,6],[0,2]])
            PAIRS = [(0, 1), (0, 2), (1, 2), (0, 0), (1, 1), (2, 2)]
            S = lambda n, m: sl(_SS + PAIRS.index((min(n, m), max(n, m))))
            x = lambda n, a: sl(_X + 2 * n + a)
            F00, F11, J, lnJ, iJ2, c1, c2 = (sl(_MISC + i) for i in range(7))
            c2l, w2, vc1 = J, iJ2, c1  # slot reuse after J/iJ2/c1 die
            t0, P1, P2, q12a, q12b = (sl(_TMP + i) for i in range(5))

            TT = nc.vector.tensor_tensor
            STT = nc.vector.scalar_tensor_tensor
            ACT = nc.scalar.activation

            import dataclasses as _dc

            def pl(base, count, step=1):
                a = sl(base)
                return _dc.replace(a, ap=[a.ap[0], [step * FREE, count], [1, FREE]])

            def bc(plane, count):
                return _dc.replace(plane, ap=[plane.ap[0], [0, count], plane.ap[1]])

            def rpl(base, count):
                # like pl() but walking slots downward (negative mid stride)
                a = sl(base)
                return _dc.replace(a, ap=[a.ap[0], [-FREE, count], [1, FREE]])

            POOL = nc.gpsimd.tensor_tensor

            def _full_body():
                # ---- 1. S'[n,m] = (G G^T)[n,m], split DVE / ACT / Pool.
                #         DVE keeps only the i=0 partial products (wave-1
                #         data) so its stream starts right after 0.6 MB of
                #         input; the i=1 partials, both partial-sum adds and
                #         all downstream S work ride the otherwise-idle Pool
                #         (GpSimd) engine, which the cost model shows runs
                #         concurrently with the DVE TT stream. ACT squares
                #         feed the diagonal as before.
                # i=0 off-diag partials land in the final S slots 38..40
                TT(out=pl(_SS, 2), in0=bc(g(0, 0), 2), in1=pl(1, 2),
                   op=ALU.mult)
                TT(out=sl(_SS + 2), in0=g(1, 0), in1=g(2, 0), op=ALU.mult)
                # diagonal squares: i=0 -> 41..43 (final), i=1 -> 58..60
                ACT(out=pl(_SS + 3, 3), in_=pl(0, 3), func=ACTF.Square)
                ACT(out=pl(_PQ, 3), in_=pl(3, 3), func=ACTF.Square)
                # Pool: i=1 off-diag partials -> 55..57, then 3-plane adds
                POOL(out=pl(_PP, 2), in0=bc(g(0, 1), 2), in1=pl(4, 2),
                     op=ALU.mult)
                POOL(out=sl(_PP + 2), in0=g(1, 1), in1=g(2, 1), op=ALU.mult)
                POOL(out=pl(_SS, 3), in0=pl(_SS, 3), in1=pl(_PP, 3),
                     op=ALU.add)
                POOL(out=pl(_SS + 3, 3), in0=pl(_SS + 3, 3), in1=pl(_PQ, 3),
                     op=ALU.add)
                # c1 depends only on q (wave 4), not on lnJ — compute it on
                # ACT up front so Pool can finish vc1 and the 6-plane S scale
                # (its longest op) early instead of stalling the DVE merged
                # +S add in steady state. vc1 gets its own slot (53): c1 is
                # still read by the stage-3 c2 add.
                ACT(out=c1, in_=q, func=ACTF.Identity, scale=ap_mu001,
                    bias=ap_mu)
                POOL(out=sl(_TMP + 3), in0=vol, in1=c1, op=ALU.mult)
                POOL(out=pl(_SS, 6), in0=pl(_SS, 6),
                     in1=bc(sl(_TMP + 3), 6), op=ALU.mult)

                # ---- 2. gradU: per column one 6-plane product (u broadcast
                #         over i), then 2-plane tree adds; temps in x region
                for cc in range(2):
                    ucc = sl(8 + cc)
                    g0 = sl(0)
                    xb = sl(_X)
                    TT(out=_dc.replace(xb, ap=[xb.ap[0], [3 * FREE, 2], [FREE, 3], [1, FREE]]),
                       in0=_dc.replace(ucc, ap=[ucc.ap[0], [0, 2], [2 * FREE, 3], [1, FREE]]),
                       in1=_dc.replace(g0, ap=[g0.ap[0], [3 * FREE, 2], [FREE, 3], [1, FREE]]),
                       op=ALU.mult)
                    TT(out=pl(_GU + 2 * cc, 2), in0=pl(_X, 2, 3),
                       in1=pl(_X + 1, 2, 3), op=ALU.add)
                    TT(out=pl(_GU + 2 * cc, 2), in0=pl(_GU + 2 * cc, 2),
                       in1=pl(_X + 2, 2, 3), op=ALU.add)
                gu00, gu01, gu10, gu11 = gu(0), gu(1), gu(2), gu(3)

                # ---- 3. F planes for ghat; cancellation-free j1 = J - 1;
                #         transcendentals on ACT (fp32 internal affine keeps
                #         the +1 exact). t0,P1 = (gu00*gu11, gu01*gu10) in
                #         one paired op (in0 slots 21,22; in1 slots 24,23).
                ACT(out=F00, in_=gu00, func=ACTF.Identity, bias=1.0)
                ACT(out=F11, in_=gu11, func=ACTF.Identity, bias=1.0)
                TT(out=pl(_TMP, 2), in0=pl(_GU, 2), in1=rpl(_GU + 3, 2), op=ALU.mult)
                TT(out=P2, in0=gu00, in1=gu11, op=ALU.add)
                TT(out=t0, in0=t0, in1=P1, op=ALU.subtract)
                TT(out=J, in0=P2, in1=t0, op=ALU.add)      # J slot holds j1
                ACT(out=lnJ, in_=J, func=ACTF.Ln, bias=1.0)
                ACT(out=iJ2, in_=lnJ, func=ACTF.Exp, scale=-2.0)
                ACT(out=c2, in_=lnJ, func=ACTF.Copy, scale=ap_nlam)
                TT(out=c2, in0=c2, in1=c1, op=ALU.add)
                # j1 dead (lnJ taken); c2l shares its slot
                ACT(out=c2l, in_=c2, func=ACTF.Identity, bias=ap_lam)

                # ---- 4. ghat = G adj(F) in three 6-plane ops: ghat(n,a) =
                #         g(n,a)*Falt - g(n,1-a)*gualt with Falt/gualt
                #         alternating-broadcast APs ((F11,F00), (gu10,gu01));
                #         G is i-major so the a-dim walks in 3-slot strides.
                g0, g3 = sl(0), sl(3)
                gho = sl(_GH)
                xb = sl(_X)
                TT(out=_dc.replace(gho, ap=[gho.ap[0], [2 * FREE, 3], [FREE, 2], [1, FREE]]),
                   in0=_dc.replace(g0, ap=[g0.ap[0], [FREE, 3], [3 * FREE, 2], [1, FREE]]),
                   in1=_dc.replace(F11, ap=[F11.ap[0], [0, 3], [-FREE, 2], [1, FREE]]),
                   op=ALU.mult)
                TT(out=_dc.replace(xb, ap=[xb.ap[0], [2 * FREE, 3], [FREE, 2], [1, FREE]]),
                   in0=_dc.replace(g3, ap=[g3.ap[0], [FREE, 3], [-3 * FREE, 2], [1, FREE]]),
                   in1=_dc.replace(gu10, ap=[gu10.ap[0], [0, 3], [-FREE, 2], [1, FREE]]),
                   op=ALU.mult)
                TT(out=pl(_GH, 6), in0=pl(_GH, 6), in1=pl(_X, 6), op=ALU.subtract)

                # ---- 5. w2 = vol * iJ2 in place (slot 29) on DVE, then one
                #         6-plane x = w2 * ghat. (vc1 and the S scale already
                #         ran on Pool back in stage 1.)
                TT(out=sl(_MISC + 4), in0=vol, in1=sl(_MISC + 4), op=ALU.mult)
                TT(out=pl(_X, 6), in0=pl(_GH, 6), in1=bc(sl(_MISC + 4), 6),
                   op=ALU.mult)

                # ---- 6. x2 = c2l x -> slots 21..26 (gu/F dead). The six
                #         diagonal products H[n,a,n,a] = x2 ghat (po 15..20)
                #         ride Pool right after x2: its ~10us there hides
                #         under the ~11us of DVE work (po 9..14 + stage 7)
                #         that sits between x2 and the merged +S add.
                X2 = 21
                TT(out=pl(X2, 6), in0=bc(c2l, 6), in1=pl(_X, 6), op=ALU.mult)
                POOL(out=pl(15, 6), in0=pl(X2, 6), in1=pl(_GH, 6),
                     op=ALU.mult)
                # a == b, n < m: H = x2[n,a] ghat[m,a] + S[n,m]  (po 9..14)
                # products: (n,m)=(0,1),(0,2) share x2[0,a] -> one 4-plane op
                x20 = sl(X2)
                p9o = sl(9)
                TT(out=_dc.replace(p9o, ap=[p9o.ap[0], [2 * FREE, 2], [FREE, 2], [1, FREE]]),
                   in0=_dc.replace(x20, ap=[x20.ap[0], [0, 2], [FREE, 2], [1, FREE]]),
                   in1=_dc.replace(sl(_GH + 2), ap=[sl(_GH + 2).ap[0], [2 * FREE, 2], [FREE, 2], [1, FREE]]),
                   op=ALU.mult)
                TT(out=pl(13, 2), in0=pl(X2 + 2, 2), in1=pl(_GH + 4, 2), op=ALU.mult)

                # ---- 7. off-diagonal pairs (po slots 0..5): three raw
                #         product pairs (P1,P2) land directly in po 2k,2k+1;
                #         one 6-plane swapped-pair tensor_scalar makes
                #         lam*(P2,P1) in slots 55..60 (Pool/ACT scratch, dead
                #         since stage 1 — S must stay live for the merged
                #         add below); then one 6-plane *c2 in place and one
                #         6-plane add.
                for k, (n, m) in enumerate(_NM):
                    TT(out=pl(2 * k, 2), in0=rpl(_X + 2 * n + 1, 2),
                       in1=pl(_GH + 2 * m, 2), op=ALU.mult)
                p1b = sl(1)
                nc.vector.tensor_scalar_mul(
                    out=pl(_PP, 6),
                    in0=_dc.replace(p1b, ap=[p1b.ap[0], [2 * FREE, 3], [-FREE, 2], [1, FREE]]),
                    scalar1=ap_lam)
                TT(out=pl(0, 6), in0=pl(0, 6), in1=bc(c2, 6), op=ALU.mult)
                TT(out=pl(0, 6), in0=pl(0, 6), in1=pl(_PP, 6), op=ALU.add)
                # wave 1: off-diagonal planes ship first
                nc.scalar.dma_start(out=hout[:, 0:6, :], in_=ar[:, 0:6, :])

                # merged adds: slots 9..20 += S, S enumerating
                # (S01,S01,S02,S02,S12,S12,S00,S00,S11,S11,S22,S22)
                p9 = sl(9)
                s0 = sl(_SS)
                TT(out=_dc.replace(p9, ap=[p9.ap[0], [2 * FREE, 6], [FREE, 2], [1, FREE]]),
                   in0=_dc.replace(p9, ap=[p9.ap[0], [2 * FREE, 6], [FREE, 2], [1, FREE]]),
                   in1=_dc.replace(s0, ap=[s0.ap[0], [FREE, 6], [0, 2], [1, FREE]]),
                   op=ALU.add)
                # wave 2: slots 9..21 (drains under stage 8)
                nc.scalar.dma_start(out=hout[:, 9:21, :], in_=ar[:, 9:21, :])

                # ---- 8. H[n,0,n,1] = x2[n,0] ghat[n,1] (po slots 6..8)
                #         computed LAST so the tail wave is only 3 planes.
                TT(out=pl(6, 3), in0=pl(X2, 3, 2), in1=pl(_GH + 1, 3, 2), op=ALU.mult)
                nc.scalar.dma_start(out=hout[:, 6:9, :], in_=ar[:, 6:9, :])

            def _dma_body():
                nc.vector.memset(ar[:, 0:21, :], 0.0)
                nc.sync.dma_start(out=hout[:, 0:21, :], in_=ar[:, 0:21, :])

            if lvl >= 3:
                _full_body()
            else:
                _dma_body()

          for _rep in range(repeat):
              _emit_rep()
    if legalize:
        _legalize_single_wait(nc)
    return nc


_NC_CACHE = None


def _get_nc():
    global _NC_CACHE
    if _NC_CACHE is None:
        _NC_CACHE = build_nc()
    return _NC_CACHE


# ------------------------------------------------------------------- host
def _shard_core(U, state, conns, shapeGrads, vols, ids, lam, mu):
    KX = len(ids)
    g6 = shapeGrads[ids, 0]                          # [KX, 3, 2] (n, i)
    fin = np.zeros((14, ELP), np.float32)
    fin[:6, :KX] = g6.transpose(2, 1, 0).reshape(6, KX)   # plane 3i+n
    fin[6, :KX] = vols[ids, 0]
    fin[7, :KX] = state[ids, 0, 0]
    uu = U[conns[ids]].reshape(KX, 6)               # (n,c) C-order
    fin[8:14, :KX] = uu.T

    mats = np.empty((PART, 4), np.float32)
    mats[:, 0] = lam
    mats[:, 1] = mu
    mats[:, 2] = 0.01 * mu
    mats[:, 3] = -lam
    return {
        "fin": np.ascontiguousarray(
            fin.reshape(14, PART, FREE).transpose(1, 0, 2)).astype(BF16NP),
        "mats": mats,
    }


_R21 = np.array([rc[0] for rc in PO_ORDER])
_C21 = np.array([rc[1] for rc in PO_ORDER])


def _decode_core(hout):
    # hout [128, 21, 784] bf16, partition-major
    planes = np.asarray(hout).astype(np.float32).transpose(1, 0, 2).reshape(21, ELP)[:, :K]
    Hm = np.empty((K, 6, 6), np.float32)
    Hm[:, _R21, _C21] = planes.T
    Hm[:, _C21[:15], _R21[:15]] = planes[:15].T     # symmetric duplicates
    return Hm.reshape(K, 3, 2, 3, 2)


def kernel(**inputs):
    U = np.asarray(inputs["U"], np.float32)
    state = np.asarray(inputs["state"], np.float32)
    conns = np.asarray(inputs["conns"])
    shapeGrads = np.asarray(inputs["shapeGrads"], np.float32)
    vols = np.asarray(inputs["vols"], np.float32)
    blocks = (np.asarray(inputs["blocks0"]), np.asarray(inputs["blocks1"]))

    core_ids = list(range(NCORES))
    in_maps = []
    id_lists = []
    for d in core_ids:
        blk, (lam, mu) = blocks[d // 4], MATS[d // 4]
        ids = blk[(d % 4) * K : (d % 4 + 1) * K]
        id_lists.append(ids)
        in_maps.append(_shard_core(U, state, conns, shapeGrads, vols, ids, lam, mu))

    res = run_bass_kernel_spmd(_get_nc(), in_maps, core_ids=core_ids)

    hess = np.empty((E, 3, 2, 3, 2), np.float32)
    for d in core_ids:
        hess[id_lists[d]] = _decode_core(res.results[d]["hout"])
    return hess

